# revision 12
# baseline (speedup 1.0000x reference)
"""Causal multi-head self-attention with RoPE on 8 TRN2 NeuronCores.

Sharding: head-parallel (16 heads -> 2 per core) for QKV projection +
attention; two per-head AllToAlls redistribute the attention output to
sequence-parallel for the output projection (each core produces a 512-row
block of the output). The head-A AllToAll overlaps head-B attention compute;
the head-A half of the output projection overlaps the head-B AllToAll.

QKV projection and all attention matmuls run in bf16 (fp32 psum
accumulate). Softmax is computed without max-subtraction (scores are O(1)
for this problem) as P = exp(S/8) * causal01; the denominator rides in the
AV matmul via a ones-column appended to V. exp/mask/score work is skipped
in the causally-dead triangle of diagonal k-chunks.

kernel(**inputs) takes the FULL unsharded inputs (x, Wqkv, Wo,
token_positions) and returns the FULL [1, 4096, 1024] output.
"""

import math
import numpy as np
from contextlib import ExitStack

import concourse.bass as bass
import concourse.tile as tile
from concourse import bacc, mybir
from concourse.bass_utils import run_bass_kernel_spmd
from concourse.masks import make_identity

F32 = mybir.dt.float32
BF16 = mybir.dt.bfloat16
AF = mybir.ActivationFunctionType
ALU = mybir.AluOpType

S = 4096          # sequence length
D = 1024          # d_model
NH = 16           # heads
DK = 64           # head dim
NCORE = 8
HPC = NH // NCORE    # 2 heads per core
DH = HPC * DK        # 128 local head dims per core
ST = 128             # s-tile (phase A)
NST = S // ST        # 32
QT = 512             # q-tile (phase B)
NQT = S // QT        # 8
KC = 128             # k-chunk
GRP = 3              # k-chunks per exp group (3 PSUM banks)
SW = 512             # s-window width for phase A xT pieces
NE = D // 128        # 8 e-chunks
THETA = 10000.0
SCALE = 1.0 / math.sqrt(DK)
SBLK = S // NCORE    # 512 output rows per core


def build():
    nc = bacc.Bacc()
    xT = nc.declare_dram_parameter("xT", [D, S], BF16, isOutput=False)
    wqkvT = nc.declare_dram_parameter("wqkvT", [D, 3 * DH], BF16, isOutput=False)
    woT = nc.declare_dram_parameter("woT", [D, D], BF16, isOutput=False)
    ce = nc.declare_dram_parameter("ce", [ST, NST * DH], F32, isOutput=False)
    se = nc.declare_dram_parameter("se", [ST, NST * DH], F32, isOutput=False)
    mask01 = nc.declare_dram_parameter("mask01", [KC, 4 * QT], BF16, isOutput=False)
    out = nc.declare_dram_parameter("out", [SBLK, D], F32, isOutput=True)

    a2a_in = [nc.dram_tensor(f"a2a_in{h}", [NCORE, DK, SBLK], BF16) for h in range(2)]
    a2a_out = [nc.dram_tensor(f"a2a_out{h}", [NCORE, DK, SBLK], BF16) for h in range(2)]
    warm_in = nc.dram_tensor("warm_in", [NCORE, 128], F32)
    warm_out = nc.dram_tensor("warm_out", [NCORE, 128], F32)

    with tile.TileContext(nc, num_cores=NCORE) as tc, ExitStack() as top:
        glob = top.enter_context(tc.tile_pool(name="glob", bufs=1))
        wpool = top.enter_context(tc.tile_pool(name="wpool", bufs=NE))

        # persistent SBUF tensors
        q_t = glob.tile([DH, S], BF16)          # [d, s]; head A rows 0:64, head B 64:128
        k_t = glob.tile([DH, S], BF16)
        v_a = glob.tile([ST, NST * (DK + 1)], BF16)   # head A V chunks + ones col
        v_b = glob.tile([ST, NST * (DK + 1)], BF16)   # head B
        attn = glob.tile([DH, S], BF16)         # attention out (unprojected), [dh_local, s]
        mask_sb = glob.tile([KC, 4 * QT], BF16)
        ident_f = glob.tile([128, 128], F32)
        ident_b = glob.tile([128, 128], BF16)

        for vdst in (v_a, v_b):
            vap = vdst[:]
            ones_view = bass.AP(tensor=vap.tensor, offset=vap.offset + DK,
                                ap=[vap.ap[0], [DK + 1, NST]])
            nc.vector.memset(ones_view, 1.0)
        make_identity(nc, ident_f[:])
        nc.vector.tensor_copy(ident_b[:], ident_f[:])

        # warmup collective: absorbs cross-core launch skew + warms ncfw while
        # phase A computes, so the real AllToAlls later are data-time only.
        nc.gpsimd.collective_compute(
            "AllToAll", ALU.bypass,
            replica_groups=[list(range(NCORE))],
            ins=[warm_in[:]], outs=[warm_out[:]],
        )

        w_sb = []
        for e in range(NE):
            w = wpool.tile([128, 3 * DH], BF16, tag="wqkv")
            nc.sync.dma_start(w[:], wqkvT[128 * e:128 * (e + 1), :])
            w_sb.append(w)
        wo_sb = [wpool.tile([128, D], BF16, tag="wo", name=f"wo{m}") for m in range(NE)]

        # ---------------- Phase A: QKV projection + RoPE + transposes ----------
        with ExitStack() as pa:
            ta = pa.enter_context(tc.tile_pool(name="ta", bufs=4))
            tbl = pa.enter_context(tc.tile_pool(name="tbl", bufs=1))
            xp = pa.enter_context(tc.tile_pool(name="xp", bufs=2 * NE))
            ps_qkv = pa.enter_context(tc.tile_pool(name="ps_qkv", bufs=3, space="PSUM"))
            ps_tr = pa.enter_context(tc.tile_pool(name="ps_tr", bufs=4, space="PSUM"))

            ce_sb = tbl.tile([ST, NST * DH], F32)
            se_sb = tbl.tile([ST, NST * DH], F32)

            for w in range(S // SW):
                pieces = []
                for e in range(NE):
                    p = xp.tile([128, SW], BF16, tag="xpiece")
                    nc.sync.dma_start(p[:], xT[128 * e:128 * (e + 1), SW * w:SW * (w + 1)])
                    pieces.append(p)
                # stream RoPE tables for this window's 4 s-tiles right behind x
                c0, c1 = DH * 4 * w, DH * 4 * (w + 1)
                nc.sync.dma_start(ce_sb[:, c0:c1], ce[:, c0:c1])
                nc.sync.dma_start(se_sb[:, c0:c1], se[:, c0:c1])
                if w == 1:
                    nc.sync.dma_start(mask_sb[:], mask01[:])
                if w == 2:
                    for m in range(NE):
                        nc.sync.dma_start(wo_sb[m][:], woT[128 * m:128 * (m + 1), :])
                for i4 in range(SW // ST):
                    i = (SW // ST) * w + i4
                    qkv_ps = ps_qkv.tile([ST, 3 * DH], F32)
                    for e in range(NE):
                        nc.tensor.matmul(
                            qkv_ps[:],
                            pieces[e][:, ST * i4:ST * (i4 + 1)],
                            w_sb[e][:],
                            start=(e == 0), stop=(e == NE - 1),
                        )
                    # RoPE on q,k columns [0:2*DH) of qkv_ps
                    qk = qkv_ps[:, 0:2 * DH]
                    # table views repeated for q and k halves
                    ce_ap = ce_sb[:, DH * i:DH * (i + 1)]
                    ce_rep = bass.AP(tensor=ce_ap.tensor, offset=ce_ap.offset,
                                     ap=[ce_ap.ap[0], [0, 2], [1, DH]])       # [p, 2, DH]
                    se_ap = se_sb[:, DH * i:DH * (i + 1)]
                    se_rep = bass.AP(tensor=se_ap.tensor, offset=se_ap.offset,
                                     ap=[se_ap.ap[0], [0, 2], [2, DK], [1, 2]])  # [p, 2, DK, 2]
                    swap_view = bass.AP(tensor=qk.tensor, offset=qk.offset + 1,
                                        ap=[qk.ap[0], [DH, 2], [2, DK], [-1, 2]])  # [p, 2, DK, 2] pair-swapped
                    qk3 = qk.rearrange("p (c f) -> p c f", c=2)
                    tmp = ta.tile([ST, 2 * DH], F32, tag="ropetmp")
                    prod = ta.tile([ST, 2 * DH], F32, tag="ropeprod")
                    qkrot = ta.tile([ST, 2 * DH], BF16, tag="qkrot")
                    nc.vector.tensor_mul(tmp[:].rearrange("p (c a b) -> p c a b", c=2, a=DK, b=2),
                                         swap_view, se_rep)
                    nc.vector.tensor_mul(prod[:].rearrange("p (c f) -> p c f", c=2), qk3, ce_rep)
                    nc.vector.tensor_add(qkrot[:], prod[:], tmp[:])
                    # transpose q and k 128-blocks into q_t / k_t (bf16);
                    # rows 0:64 land on head A, 64:128 on head B directly.
                    for part, dst in ((0, q_t), (1, k_t)):
                        tr = ps_tr.tile([128, 128], BF16, tag="tr")
                        nc.tensor.transpose(tr[:],
                                            qkrot[:, DH * part:DH * (part + 1)],
                                            ident_b[:])
                        nc.vector.tensor_copy(dst[:, ST * i:ST * (i + 1)], tr[:])
                    # V chunks + ones column
                    for h, vdst in ((0, v_a), (1, v_b)):
                        base = (DK + 1) * i
                        nc.vector.tensor_copy(vdst[:, base:base + DK],
                                              qkv_ps[:, 2 * DH + DK * h:2 * DH + DK * (h + 1)])

        # ---------------- Phase B: attention (head A fully, then head B) -------
        with ExitStack() as pb:
            pp = pb.enter_context(tc.tile_pool(name="pp", bufs=6))
            nrm = pb.enter_context(tc.tile_pool(name="nrm", bufs=2))
            ps_s = pb.enter_context(tc.tile_pool(name="ps_s", bufs=2, space="PSUM"))
            ps_o = pb.enter_context(tc.tile_pool(name="ps_o", bufs=2, space="PSUM"))

            def qlo_of(j, kc):
                d2 = kc - 4 * j
                return KC * d2 if 0 <= d2 < 4 else 0

            for h in (0, 1):
                q_h = q_t[DK * h:DK * (h + 1), :]
                k_h = k_t[DK * h:DK * (h + 1), :]
                v_h = v_a if h == 0 else v_b
                for j in range(NQT):
                    nk = 4 * (j + 1)
                    o_ps = ps_o.tile([DK + 1, QT], F32, tag="ops", name="ops")
                    ngrp = (nk + GRP - 1) // GRP
                    for g in range(ngrp):
                        chunks = list(range(g * GRP, min((g + 1) * GRP, nk)))
                        s_ps = ps_s.tile([KC, GRP * QT], F32, tag="sgrp", name="sgrp")
                        for idx, kc in enumerate(chunks):
                            # diagonal chunks: scores/exp below q=128*d2 never read
                            sq = qlo_of(j, kc)
                            nc.tensor.ldweights(k_h[:, KC * kc:KC * (kc + 1)],
                                                tile_position=(DK * h, 0))
                            _mm = nc.tensor.matmul(
                                s_ps[:, QT * idx + sq:QT * (idx + 1)],
                                k_h[:, KC * kc:KC * (kc + 1)],
                                q_h[:, QT * j + sq:QT * (j + 1)],
                                start=True, stop=True, skip_group_check=True,
                            )
                            _mm.ins.ldweights = False
                        pg = pp.tile([KC, GRP * QT], BF16, tag="pgrp", name="pgrp")
                        n = len(chunks)
                        idx = 0
                        while idx < n:
                            if qlo_of(j, chunks[idx]) == 0:
                                end = idx
                                while end < n and qlo_of(j, chunks[end]) == 0:
                                    end += 1
                                nc.scalar.activation(pg[:, QT * idx:QT * end],
                                                     s_ps[:, QT * idx:QT * end],
                                                     AF.Exp, scale=float(SCALE))
                                idx = end
                            else:
                                q0 = QT * idx + qlo_of(j, chunks[idx])
                                nc.scalar.activation(pg[:, q0:QT * (idx + 1)],
                                                     s_ps[:, q0:QT * (idx + 1)],
                                                     AF.Exp, scale=float(SCALE))
                                idx += 1
                        # causal 0/1 mask on gpsimd: keeps the vector engine off
                        # the exp->AV critical path
                        for idx, kc in enumerate(chunks):
                            d2 = kc - 4 * j
                            if 0 <= d2 < 4:
                                ql = KC * d2
                                nc.gpsimd.tensor_mul(pg[:, QT * idx + ql:QT * (idx + 1)],
                                                     pg[:, QT * idx + ql:QT * (idx + 1)],
                                                     mask_sb[:, QT * d2 + ql:QT * (d2 + 1)])
                        for idx, kc in enumerate(chunks):
                            d2 = kc - 4 * j
                            ql = KC * d2 if d2 in (1, 2, 3) else 0
                            nc.tensor.ldweights(v_h[:, (DK + 1) * kc:(DK + 1) * (kc + 1)])
                            _mm = nc.tensor.matmul(
                                o_ps[:, ql:QT],
                                v_h[:, (DK + 1) * kc:(DK + 1) * (kc + 1)],
                                pg[:, QT * idx + ql:QT * (idx + 1)],
                                start=(kc == 0), stop=(kc == nk - 1),
                                skip_group_check=True,
                            )
                            _mm.ins.ldweights = False
                    rec = nrm.tile([1, QT], F32, tag="rec", name="rec")
                    bc = nrm.tile([DK, QT], F32, tag="bc", name="bc")
                    nc.vector.reciprocal(rec[0:1, :], o_ps[DK:DK + 1, :])
                    nc.gpsimd.partition_broadcast(bc[0:DK, :], rec[0:1, :])
                    nc.vector.tensor_mul(attn[DK * h:DK * (h + 1), QT * j:QT * (j + 1)],
                                         o_ps[0:DK, :], bc[0:DK, :])
                    nc.sync.dma_start(a2a_in[h][j, :, :],
                                      attn[DK * h:DK * (h + 1), SBLK * j:SBLK * (j + 1)])
                nc.gpsimd.collective_compute(
                    "AllToAll", ALU.bypass,
                    replica_groups=[list(range(NCORE))],
                    ins=[a2a_in[h][:]], outs=[a2a_out[h][:]],
                )

        # ---------------- Phase D: output projection ---------------------------
        with ExitStack() as pd:
            gpool = pd.enter_context(tc.tile_pool(name="gpool", bufs=NCORE))
            opool = pd.enter_context(tc.tile_pool(name="opool", bufs=2))
            ps_d = pd.enter_context(tc.tile_pool(name="ps_d", bufs=2, space="PSUM"))

            g_sb = [gpool.tile([DH, SBLK], BF16, tag="gath", name=f"gath{m}") for m in range(NCORE)]
            for m in range(NCORE):
                nc.sync.dma_start(g_sb[m][0:DK, :], a2a_out[0][m, :, :])
            for m in range(NCORE):
                nc.sync.dma_start(g_sb[m][DK:DH, :], a2a_out[1][m, :, :])
            for t in range(SBLK // 128):
                op_ps = ps_d.tile([128, D], F32, tag="dps", name="dps")
                for m in range(NCORE):
                    nc.tensor.ldweights(g_sb[m][:, 128 * t:128 * (t + 1)])
                    for e2 in range(2):
                        _mm = nc.tensor.matmul(
                            op_ps[:, 512 * e2:512 * (e2 + 1)],
                            g_sb[m][:, 128 * t:128 * (t + 1)],
                            wo_sb[m][:, 512 * e2:512 * (e2 + 1)],
                            start=(m == 0), stop=(m == NCORE - 1),
                            skip_group_check=True,
                        )
                        _mm.ins.ldweights = False
                o_sb = opool.tile([128, D], F32, tag="osb", name="osb")
                nc.vector.tensor_copy(o_sb[:], op_ps[:])
                nc.sync.dma_start(out[128 * t:128 * (t + 1), :], o_sb[:])

    nc.compile()
    return nc


_NC = None
TRACE = False
LAST_EXEC_NS = None


def _host_inputs(x, Wqkv, Wo, token_positions):
    """Build per-core input maps (slicing + layout prep only)."""
    import ml_dtypes
    x = np.asarray(x, dtype=np.float32).reshape(S, D)
    Wqkv = np.asarray(Wqkv, dtype=np.float32)
    Wo = np.asarray(Wo, dtype=np.float32)
    pos = np.asarray(token_positions).astype(np.float32)

    xT = np.ascontiguousarray(x.T).astype(ml_dtypes.bfloat16)   # [D, S]
    woT_full = np.ascontiguousarray(Wo.T)               # [dh_global, e]

    # RoPE tables, [ST, NST*DH] tiled: block i holds rows 128i..128i+127
    kd = np.arange(0, DK, 2, dtype=np.float32) / np.float32(DK)
    inv = np.float32(THETA) ** kd                       # [32]
    ang = pos[:, None] / inv[None, :]                   # [S, 32] f32
    cos = np.cos(ang.astype(np.float64)).astype(np.float32)
    sin = np.sin(ang.astype(np.float64)).astype(np.float32)
    ce64 = np.repeat(cos, 2, axis=1)                    # [S, 64]
    se64 = np.empty((S, DK), dtype=np.float32)
    se64[:, 0::2] = -sin
    se64[:, 1::2] = sin
    ce128 = np.concatenate([ce64, ce64], axis=1)        # [S, 128] two heads
    se128 = np.concatenate([se64, se64], axis=1)
    ce_t = np.ascontiguousarray(ce128.reshape(NST, ST, DH).transpose(1, 0, 2).reshape(ST, NST * DH))
    se_t = np.ascontiguousarray(se128.reshape(NST, ST, DH).transpose(1, 0, 2).reshape(ST, NST * DH))

    # causal 0/1 mask for the 4 diagonal chunks: [128, 4*512]
    p = np.arange(KC)[:, None]
    m = np.empty((KC, 4 * QT), dtype=np.float32)
    for c2 in range(4):
        ql = np.arange(QT)[None, :]
        m[:, QT * c2:QT * (c2 + 1)] = (ql >= KC * c2 + p).astype(np.float32)

    in_maps = []
    for core in range(NCORE):
        r0 = DH * core
        wq = Wqkv[r0:r0 + DH]
        wk = Wqkv[D + r0:D + r0 + DH]
        wv = Wqkv[2 * D + r0:2 * D + r0 + DH]
        wqkvT = np.ascontiguousarray(np.concatenate([wq, wk, wv], axis=0).T)  # [D, 384]
        in_maps.append({
            "xT": xT,
            "wqkvT": wqkvT.astype(ml_dtypes.bfloat16),
            "woT": woT_full.astype(ml_dtypes.bfloat16),
            "ce": ce_t,
            "se": se_t,
            "mask01": m.astype(ml_dtypes.bfloat16),
        })
    return in_maps


def kernel(x, Wqkv, Wo, token_positions):
    global _NC, LAST_EXEC_NS
    if _NC is None:
        _NC = build()
    in_maps = _host_inputs(x, Wqkv, Wo, token_positions)
    kwargs = {}
    if TRACE:
        import tempfile
        kwargs = {"trace": True, "tmpdir": tempfile.mkdtemp(prefix="attn_trace_")}
        if TRACE == "all":
            kwargs["trace_cores"] = list(range(NCORE))
        print("trace dir:", kwargs["tmpdir"])
    res = run_bass_kernel_spmd(_NC, in_maps, list(range(NCORE)), **kwargs)
    LAST_EXEC_NS = res.exec_time_ns
    out = np.concatenate([res.results[c]["out"] for c in range(NCORE)], axis=0)
    return out.reshape(1, S, D)


# revision 13
# speedup vs baseline: 1.2866x; 1.2866x over previous
"""Causal multi-head self-attention with RoPE on 8 TRN2 NeuronCores.

Sharding: head-parallel (16 heads -> 2 per core) for QKV projection +
attention; two per-head AllToAlls redistribute the attention output to
sequence-parallel for the output projection (each core produces a 512-row
block of the output). The head-A AllToAll overlaps head-B attention compute;
the head-A half of the output projection overlaps the head-B AllToAll.

QKV projection and all attention matmuls run in bf16 (fp32 psum
accumulate). Softmax is computed without max-subtraction (scores are O(1)
for this problem) as P = exp(S/8) * causal01; the denominator rides in the
AV matmul via a ones-column appended to V. exp/mask/score work is skipped
in the causally-dead triangle of diagonal k-chunks.

kernel(**inputs) takes the FULL unsharded inputs (x, Wqkv, Wo,
token_positions) and returns the FULL [1, 4096, 1024] output.
"""

import math
import numpy as np
from contextlib import ExitStack

import concourse.bass as bass
import concourse.tile as tile
from concourse import bacc, mybir
from concourse.bass_utils import run_bass_kernel_spmd
from concourse.masks import make_identity

F32 = mybir.dt.float32
BF16 = mybir.dt.bfloat16
AF = mybir.ActivationFunctionType
ALU = mybir.AluOpType

S = 4096          # sequence length
D = 1024          # d_model
NH = 16           # heads
DK = 64           # head dim
NCORE = 8
HPC = NH // NCORE    # 2 heads per core
DH = HPC * DK        # 128 local head dims per core
ST = 128             # s-tile (phase A)
NST = S // ST        # 32
QT = 512             # q-tile (phase B)
NQT = S // QT        # 8
KC = 128             # k-chunk
GRP = 3              # k-chunks per exp group (3 PSUM banks)
SW = 512             # s-window width for phase A xT pieces
NE = D // 128        # 8 e-chunks
THETA = 10000.0
SCALE = 1.0 / math.sqrt(DK)
SBLK = S // NCORE    # 512 output rows per core


def build():
    nc = bacc.Bacc()
    xT = nc.declare_dram_parameter("xT", [D, S], BF16, isOutput=False)
    wqkvT = nc.declare_dram_parameter("wqkvT", [D, 3 * DH], BF16, isOutput=False)
    woT = nc.declare_dram_parameter("woT", [D, D], BF16, isOutput=False)
    ce = nc.declare_dram_parameter("ce", [ST, NST * DH], F32, isOutput=False)
    se = nc.declare_dram_parameter("se", [ST, NST * DH], F32, isOutput=False)
    mask01 = nc.declare_dram_parameter("mask01", [KC, 4 * QT], BF16, isOutput=False)
    out = nc.declare_dram_parameter("out", [SBLK, D], F32, isOutput=True)

    a2a_in = [nc.dram_tensor(f"a2a_in{h}", [NCORE, DK, SBLK], BF16) for h in range(2)]
    a2a_out = [nc.dram_tensor(f"a2a_out{h}", [NCORE, DK, SBLK], BF16) for h in range(2)]
    warm_in = nc.dram_tensor("warm_in", [NCORE, 128], F32)
    warm_out = nc.dram_tensor("warm_out", [NCORE, 128], F32)

    with tile.TileContext(nc, num_cores=NCORE) as tc, ExitStack() as top:
        glob = top.enter_context(tc.tile_pool(name="glob", bufs=1))
        wpool = top.enter_context(tc.tile_pool(name="wpool", bufs=NE))

        # persistent SBUF tensors
        q_t = glob.tile([DH, S], BF16)          # [d, s]; head A rows 0:64, head B 64:128
        k_t = glob.tile([DH, S], BF16)
        v_a = glob.tile([ST, NST * (DK + 1)], BF16)   # head A V chunks + ones col
        v_b = glob.tile([ST, NST * (DK + 1)], BF16)   # head B
        attn = glob.tile([DH, S], BF16)         # attention out (unprojected), [dh_local, s]
        mask_sb = glob.tile([KC, 4 * QT], BF16)
        ident_f = glob.tile([128, 128], F32)
        ident_b = glob.tile([128, 128], BF16)

        for vdst in (v_a, v_b):
            vap = vdst[:]
            ones_view = bass.AP(tensor=vap.tensor, offset=vap.offset + DK,
                                ap=[vap.ap[0], [DK + 1, NST]])
            nc.vector.memset(ones_view, 1.0)
        make_identity(nc, ident_f[:])
        nc.vector.tensor_copy(ident_b[:], ident_f[:])

        # warmup collective: absorbs cross-core launch skew + warms ncfw while
        # phase A computes, so the real AllToAlls later are data-time only.
        nc.gpsimd.collective_compute(
            "AllToAll", ALU.bypass,
            replica_groups=[list(range(NCORE))],
            ins=[warm_in[:]], outs=[warm_out[:]],
        )

        w_sb = []
        for e in range(NE):
            w = wpool.tile([128, 3 * DH], BF16, tag="wqkv")
            nc.sync.dma_start(w[:], wqkvT[128 * e:128 * (e + 1), :])
            w_sb.append(w)
        wo_sb = [wpool.tile([128, D], BF16, tag="wo", name=f"wo{m}") for m in range(NE)]

        # ---------------- Phase A: QKV projection + RoPE + transposes ----------
        with ExitStack() as pa:
            ta = pa.enter_context(tc.tile_pool(name="ta", bufs=4))
            tbl = pa.enter_context(tc.tile_pool(name="tbl", bufs=1))
            xp = pa.enter_context(tc.tile_pool(name="xp", bufs=2 * NE))
            ps_qkv = pa.enter_context(tc.tile_pool(name="ps_qkv", bufs=3, space="PSUM"))
            ps_tr = pa.enter_context(tc.tile_pool(name="ps_tr", bufs=4, space="PSUM"))

            ce_sb = tbl.tile([ST, NST * DH], F32)
            se_sb = tbl.tile([ST, NST * DH], F32)

            for w in range(S // SW):
                pieces = []
                for e in range(NE):
                    p = xp.tile([128, SW], BF16, tag="xpiece")
                    nc.sync.dma_start(p[:], xT[128 * e:128 * (e + 1), SW * w:SW * (w + 1)])
                    pieces.append(p)
                # stream RoPE tables for this window's 4 s-tiles right behind x
                c0, c1 = DH * 4 * w, DH * 4 * (w + 1)
                nc.sync.dma_start(ce_sb[:, c0:c1], ce[:, c0:c1])
                nc.sync.dma_start(se_sb[:, c0:c1], se[:, c0:c1])
                if w == 1:
                    nc.sync.dma_start(mask_sb[:], mask01[:])
                if w == 2:
                    for m in range(NE):
                        nc.sync.dma_start(wo_sb[m][:], woT[128 * m:128 * (m + 1), :])
                for i4 in range(SW // ST):
                    i = (SW // ST) * w + i4
                    qkv_ps = ps_qkv.tile([ST, 3 * DH], F32)
                    for e in range(NE):
                        nc.tensor.matmul(
                            qkv_ps[:],
                            pieces[e][:, ST * i4:ST * (i4 + 1)],
                            w_sb[e][:],
                            start=(e == 0), stop=(e == NE - 1),
                        )
                    # RoPE on q,k columns [0:2*DH) of qkv_ps
                    qk = qkv_ps[:, 0:2 * DH]
                    # table views repeated for q and k halves
                    ce_ap = ce_sb[:, DH * i:DH * (i + 1)]
                    ce_rep = bass.AP(tensor=ce_ap.tensor, offset=ce_ap.offset,
                                     ap=[ce_ap.ap[0], [0, 2], [1, DH]])       # [p, 2, DH]
                    se_ap = se_sb[:, DH * i:DH * (i + 1)]
                    se_rep = bass.AP(tensor=se_ap.tensor, offset=se_ap.offset,
                                     ap=[se_ap.ap[0], [0, 2], [2, DK], [1, 2]])  # [p, 2, DK, 2]
                    swap_view = bass.AP(tensor=qk.tensor, offset=qk.offset + 1,
                                        ap=[qk.ap[0], [DH, 2], [2, DK], [-1, 2]])  # [p, 2, DK, 2] pair-swapped
                    qk3 = qk.rearrange("p (c f) -> p c f", c=2)
                    tmp = ta.tile([ST, 2 * DH], F32, tag="ropetmp")
                    prod = ta.tile([ST, 2 * DH], F32, tag="ropeprod")
                    qkrot = ta.tile([ST, 2 * DH], BF16, tag="qkrot")
                    nc.vector.tensor_mul(tmp[:].rearrange("p (c a b) -> p c a b", c=2, a=DK, b=2),
                                         swap_view, se_rep)
                    nc.vector.tensor_mul(prod[:].rearrange("p (c f) -> p c f", c=2), qk3, ce_rep)
                    nc.vector.tensor_add(qkrot[:], prod[:], tmp[:])
                    # transpose q and k 128-blocks into q_t / k_t (bf16);
                    # rows 0:64 land on head A, 64:128 on head B directly.
                    for part, dst in ((0, q_t), (1, k_t)):
                        tr = ps_tr.tile([128, 128], BF16, tag="tr")
                        nc.tensor.transpose(tr[:],
                                            qkrot[:, DH * part:DH * (part + 1)],
                                            ident_b[:])
                        nc.vector.tensor_copy(dst[:, ST * i:ST * (i + 1)], tr[:])
                    # V chunks + ones column
                    for h, vdst in ((0, v_a), (1, v_b)):
                        base = (DK + 1) * i
                        nc.vector.tensor_copy(vdst[:, base:base + DK],
                                              qkv_ps[:, 2 * DH + DK * h:2 * DH + DK * (h + 1)])

        # ---------------- Phase B: attention (head A fully, then head B) -------
        with ExitStack() as pb:
            pp = pb.enter_context(tc.tile_pool(name="pp", bufs=6))
            nrm = pb.enter_context(tc.tile_pool(name="nrm", bufs=2))
            ps_s = pb.enter_context(tc.tile_pool(name="ps_s", bufs=2, space="PSUM"))
            ps_o = pb.enter_context(tc.tile_pool(name="ps_o", bufs=2, space="PSUM"))

            def qlo_of(j, kc):
                d2 = kc - 4 * j
                return KC * d2 if 0 <= d2 < 4 else 0

            for h in (0, 1):
                q_h = q_t[DK * h:DK * (h + 1), :]
                k_h = k_t[DK * h:DK * (h + 1), :]
                v_h = v_a if h == 0 else v_b
                for j in range(NQT):
                    nk = 4 * (j + 1)
                    o_ps = ps_o.tile([DK + 1, QT], F32, tag="ops", name="ops")
                    ngrp = (nk + GRP - 1) // GRP
                    for g in range(ngrp):
                        chunks = list(range(g * GRP, min((g + 1) * GRP, nk)))
                        s_ps = ps_s.tile([KC, GRP * QT], F32, tag="sgrp", name="sgrp")
                        for idx, kc in enumerate(chunks):
                            # diagonal chunks: scores/exp below q=128*d2 never read
                            sq = qlo_of(j, kc)
                            nc.tensor.ldweights(k_h[:, KC * kc:KC * (kc + 1)],
                                                tile_position=(DK * h, 0))
                            _mm = nc.tensor.matmul(
                                s_ps[:, QT * idx + sq:QT * (idx + 1)],
                                k_h[:, KC * kc:KC * (kc + 1)],
                                q_h[:, QT * j + sq:QT * (j + 1)],
                                start=True, stop=True, skip_group_check=True,
                            )
                            _mm.ins.ldweights = False
                        pg = pp.tile([KC, GRP * QT], BF16, tag="pgrp", name="pgrp")
                        n = len(chunks)
                        idx = 0
                        while idx < n:
                            if qlo_of(j, chunks[idx]) == 0:
                                end = idx
                                while end < n and qlo_of(j, chunks[end]) == 0:
                                    end += 1
                                nc.scalar.activation(pg[:, QT * idx:QT * end],
                                                     s_ps[:, QT * idx:QT * end],
                                                     AF.Exp, scale=float(SCALE))
                                idx = end
                            else:
                                q0 = QT * idx + qlo_of(j, chunks[idx])
                                nc.scalar.activation(pg[:, q0:QT * (idx + 1)],
                                                     s_ps[:, q0:QT * (idx + 1)],
                                                     AF.Exp, scale=float(SCALE))
                                idx += 1
                        for idx, kc in enumerate(chunks):
                            d2 = kc - 4 * j
                            if 0 <= d2 < 4:
                                ql = KC * d2
                                nc.vector.tensor_mul(pg[:, QT * idx + ql:QT * (idx + 1)],
                                                     pg[:, QT * idx + ql:QT * (idx + 1)],
                                                     mask_sb[:, QT * d2 + ql:QT * (d2 + 1)])
                        for idx, kc in enumerate(chunks):
                            d2 = kc - 4 * j
                            ql = KC * d2 if d2 in (1, 2, 3) else 0
                            nc.tensor.ldweights(v_h[:, (DK + 1) * kc:(DK + 1) * (kc + 1)])
                            _mm = nc.tensor.matmul(
                                o_ps[:, ql:QT],
                                v_h[:, (DK + 1) * kc:(DK + 1) * (kc + 1)],
                                pg[:, QT * idx + ql:QT * (idx + 1)],
                                start=(kc == 0), stop=(kc == nk - 1),
                                skip_group_check=True,
                            )
                            _mm.ins.ldweights = False
                    rec = nrm.tile([1, QT], F32, tag="rec", name="rec")
                    bc = nrm.tile([DK, QT], F32, tag="bc", name="bc")
                    nc.vector.reciprocal(rec[0:1, :], o_ps[DK:DK + 1, :])
                    nc.gpsimd.partition_broadcast(bc[0:DK, :], rec[0:1, :])
                    nc.vector.tensor_mul(attn[DK * h:DK * (h + 1), QT * j:QT * (j + 1)],
                                         o_ps[0:DK, :], bc[0:DK, :])
                    nc.sync.dma_start(a2a_in[h][j, :, :],
                                      attn[DK * h:DK * (h + 1), SBLK * j:SBLK * (j + 1)])
                nc.gpsimd.collective_compute(
                    "AllToAll", ALU.bypass,
                    replica_groups=[list(range(NCORE))],
                    ins=[a2a_in[h][:]], outs=[a2a_out[h][:]],
                )

        # ---------------- Phase D: output projection ---------------------------
        with ExitStack() as pd:
            gpool = pd.enter_context(tc.tile_pool(name="gpool", bufs=NCORE))
            opool = pd.enter_context(tc.tile_pool(name="opool", bufs=2))
            ps_d = pd.enter_context(tc.tile_pool(name="ps_d", bufs=2, space="PSUM"))

            g_sb = [gpool.tile([DH, SBLK], BF16, tag="gath", name=f"gath{m}") for m in range(NCORE)]
            for m in range(NCORE):
                nc.sync.dma_start(g_sb[m][0:DK, :], a2a_out[0][m, :, :])
            for m in range(NCORE):
                nc.sync.dma_start(g_sb[m][DK:DH, :], a2a_out[1][m, :, :])
            for t in range(SBLK // 128):
                op_ps = ps_d.tile([128, D], F32, tag="dps", name="dps")
                for m in range(NCORE):
                    nc.tensor.ldweights(g_sb[m][:, 128 * t:128 * (t + 1)])
                    for e2 in range(2):
                        _mm = nc.tensor.matmul(
                            op_ps[:, 512 * e2:512 * (e2 + 1)],
                            g_sb[m][:, 128 * t:128 * (t + 1)],
                            wo_sb[m][:, 512 * e2:512 * (e2 + 1)],
                            start=(m == 0), stop=(m == NCORE - 1),
                            skip_group_check=True,
                        )
                        _mm.ins.ldweights = False
                o_sb = opool.tile([128, D], F32, tag="osb", name="osb")
                nc.vector.tensor_copy(o_sb[:], op_ps[:])
                nc.sync.dma_start(out[128 * t:128 * (t + 1), :], o_sb[:])

    nc.compile()
    return nc


_NC = None
TRACE = False
LAST_EXEC_NS = None


def _host_inputs(x, Wqkv, Wo, token_positions):
    """Build per-core input maps (slicing + layout prep only)."""
    import ml_dtypes
    x = np.asarray(x, dtype=np.float32).reshape(S, D)
    Wqkv = np.asarray(Wqkv, dtype=np.float32)
    Wo = np.asarray(Wo, dtype=np.float32)
    pos = np.asarray(token_positions).astype(np.float32)

    xT = np.ascontiguousarray(x.T).astype(ml_dtypes.bfloat16)   # [D, S]
    woT_full = np.ascontiguousarray(Wo.T)               # [dh_global, e]

    # RoPE tables, [ST, NST*DH] tiled: block i holds rows 128i..128i+127
    kd = np.arange(0, DK, 2, dtype=np.float32) / np.float32(DK)
    inv = np.float32(THETA) ** kd                       # [32]
    ang = pos[:, None] / inv[None, :]                   # [S, 32] f32
    cos = np.cos(ang.astype(np.float64)).astype(np.float32)
    sin = np.sin(ang.astype(np.float64)).astype(np.float32)
    ce64 = np.repeat(cos, 2, axis=1)                    # [S, 64]
    se64 = np.empty((S, DK), dtype=np.float32)
    se64[:, 0::2] = -sin
    se64[:, 1::2] = sin
    ce128 = np.concatenate([ce64, ce64], axis=1)        # [S, 128] two heads
    se128 = np.concatenate([se64, se64], axis=1)
    ce_t = np.ascontiguousarray(ce128.reshape(NST, ST, DH).transpose(1, 0, 2).reshape(ST, NST * DH))
    se_t = np.ascontiguousarray(se128.reshape(NST, ST, DH).transpose(1, 0, 2).reshape(ST, NST * DH))

    # causal 0/1 mask for the 4 diagonal chunks: [128, 4*512]
    p = np.arange(KC)[:, None]
    m = np.empty((KC, 4 * QT), dtype=np.float32)
    for c2 in range(4):
        ql = np.arange(QT)[None, :]
        m[:, QT * c2:QT * (c2 + 1)] = (ql >= KC * c2 + p).astype(np.float32)

    in_maps = []
    for core in range(NCORE):
        r0 = DH * core
        wq = Wqkv[r0:r0 + DH]
        wk = Wqkv[D + r0:D + r0 + DH]
        wv = Wqkv[2 * D + r0:2 * D + r0 + DH]
        wqkvT = np.ascontiguousarray(np.concatenate([wq, wk, wv], axis=0).T)  # [D, 384]
        in_maps.append({
            "xT": xT,
            "wqkvT": wqkvT.astype(ml_dtypes.bfloat16),
            "woT": woT_full.astype(ml_dtypes.bfloat16),
            "ce": ce_t,
            "se": se_t,
            "mask01": m.astype(ml_dtypes.bfloat16),
        })
    return in_maps


def kernel(x, Wqkv, Wo, token_positions):
    global _NC, LAST_EXEC_NS
    if _NC is None:
        _NC = build()
    in_maps = _host_inputs(x, Wqkv, Wo, token_positions)
    kwargs = {}
    if TRACE:
        import tempfile
        kwargs = {"trace": True, "tmpdir": tempfile.mkdtemp(prefix="attn_trace_")}
        if TRACE == "all":
            kwargs["trace_cores"] = list(range(NCORE))
        print("trace dir:", kwargs["tmpdir"])
    res = run_bass_kernel_spmd(_NC, in_maps, list(range(NCORE)), **kwargs)
    LAST_EXEC_NS = res.exec_time_ns
    out = np.concatenate([res.results[c]["out"] for c in range(NCORE)], axis=0)
    return out.reshape(1, S, D)
